# revision 1
# baseline (speedup 1.0000x reference)
"""Trainium2 Bass kernel for nn_DeformableTemporalBlock — v2.

Data-parallel over batch M=8 (one sample per core). Major changes vs v1:
  - host supplies x already transposed ([B,T]) and bf16; stage-0 transposes
    gone; residual add + GEMM2 run in [B,T] layout (host re-transposes out).
  - bf16 datapath for p/xp/q and all matmuls (PSUM stays f32): DVE gets
    2-4x perf modes, PE gets FWL weight loads; f32 only for offsets/G-build
    where positions up to ~127 need mantissa.
  - GEMM2 emitted as [B,T] (N=512 per matmul) instead of [T,B] (N=128,
    which pays the fp32r 4x small-N penalty).
  - G-matrix broadcast via PE rank-1 matmul instead of gpsimd
    partition_broadcast; offset+tmod fold done globally in 2 DVE ops.
  - sampling chunks processed 4 per PSUM group; combine = 1 ACT + 2 DVE
    ops; prelu2 moved to DVE (2 bf16 ops); squares accumulate on DVE.
  - gLN1 sum(p) via PE ones-column reduction instead of ACT accum reads.
"""

import os
import sys
import functools
import numpy as np

for _p in ("/opt/trn_rl_repo", "/root/.axon_site/_ro/trn_rl_repo"):
    if _p not in sys.path and os.path.isdir(_p):
        sys.path.append(_p)

import ml_dtypes
import concourse.bass as bass
import concourse.mybir as mybir
from concourse import bacc, tile, library_config
from concourse.bass_utils import run_bass_kernel_spmd

F32 = mybir.dt.float32
BF16 = mybir.dt.bfloat16
AF = mybir.ActivationFunctionType
ALU = mybir.AluOpType
AX = mybir.AxisListType
BF = ml_dtypes.bfloat16

T, B, H, KK = 4000, 128, 512, 3
NH = H // 128          # 4 h-tiles
TC = 123               # t-chunk size (window = TC+5 = 128 = one K tile)
NC = 33                # ceil(T/TC); NC*TC = 4059
TP = NC * TC           # padded t extent 4059
GC = 3                 # chunks per sampling group
NG = (NC + GC - 1) // GC   # 9 groups
SW = 3 * TC + 3        # 372: per-chunk G width incl sentinel cols
GW = GC * SW           # 1488
NT = float(T * H)      # gLN element count
EPS = 1e-8
NCORES = 8


def _build(a1, aodc, aopc, a2):
    nc = bacc.Bacc("TRN2", target_bir_lowering=False, debug=False)

    def din(name, shape, dt=F32):
        return nc.dram_tensor(name, list(shape), dt, kind="ExternalInput")

    xt_d = din("xt", (B, T), BF16)
    w1t_d = din("w1t", (B, H), BF16)
    id_d = din("id128", (128, 128), BF16)
    niota_d = din("niota", (128, 1))
    ones128_d = din("ones128", (128, 1))
    onesrow_d = din("onesrow", (1, 128))
    onescol_d = din("onescol", (128, 1), BF16)
    tmod_d = din("tmodTP", (KK, TP))
    dil_d = din("dil3", (KK, 1))
    offdw_d = din("offdw12", (128, 12))
    swo_d = din("swo4", (128, 4))
    offpw_d = din("offpwT12", (128, 12), BF16)
    dww_d = din("dww12", (128, 12))
    swdw_d = din("swdw4", (128, 4))
    dwb_d = din("dwb4", (128, 4))
    g1_d = din("g1t4", (128, 4))
    b1_d = din("b1t4", (128, 4))
    g2_d = din("g2t4", (128, 4))
    b2_d = din("b2t4", (128, 4))
    pwt_d = din("pwT4", (128, H))
    pwg_d = din("pwG4", (128, H), BF16)
    sent_d = din("sentGW", (1, GW))
    out_d = nc.dram_tensor("out", [B, T], BF16, kind="ExternalOutput")

    with tile.TileContext(nc) as tc:
        with tc.tile_pool(name="persist", bufs=1) as pp, \
             tc.tile_pool(name="work", bufs=3) as wk, \
             tc.tile_pool(name="ps", bufs=2, space="PSUM") as ps, \
             tc.tile_pool(name="pspin", bufs=1, space="PSUM") as pspin, \
             tc.tile_pool(name="sm", bufs=1, space="PSUM") as psm:
            nc.gpsimd.load_library(library_config.mlp)

            def f32r(ap):
                return ap.bitcast(mybir.dt.float32r)

            # ---- persistent SBUF ----
            xt = pp.tile([B, T], BF16, tag="xt")
            w1t = pp.tile([B, H], BF16, tag="w1t")
            id128 = pp.tile([128, 128], BF16, tag="id128")
            niota = pp.tile([128, 1], F32, tag="niota")
            ones128 = pp.tile([128, 1], F32, tag="ones128")
            onesrow = pp.tile([1, 128], F32, tag="onesrow")
            onescol = pp.tile([128, 1], BF16, tag="onescol")
            tmod = pp.tile([KK, TP], F32, tag="tmod")
            dil = pp.tile([KK, 1], F32, tag="dil")
            offdw = pp.tile([128, 12], F32, tag="offdw")
            swo = pp.tile([128, 4], F32, tag="swo")
            offpw = pp.tile([128, 12], BF16, tag="offpw")
            dww = pp.tile([128, 12], F32, tag="dww")
            swdw = pp.tile([128, 4], F32, tag="swdw")
            dwb = pp.tile([128, 4], F32, tag="dwb")
            g1w = pp.tile([128, 4], F32, tag="g1w")
            b1w = pp.tile([128, 4], F32, tag="b1w")
            g2w = pp.tile([128, 4], F32, tag="g2w")
            b2w = pp.tile([128, 4], F32, tag="b2w")
            pwt = pp.tile([128, H], F32, tag="pwt")
            pwg = pp.tile([128, H], BF16, tag="pwg")

            for t_, d_ in ((xt, xt_d), (w1t, w1t_d), (id128, id_d),
                           (niota, niota_d), (ones128, ones128_d),
                           (onesrow, onesrow_d), (onescol, onescol_d),
                           (tmod, tmod_d), (dil, dil_d), (offdw, offdw_d),
                           (swo, swo_d), (offpw, offpw_d), (dww, dww_d),
                           (swdw, swdw_d), (dwb, dwb_d), (g1w, g1_d),
                           (b1w, b1_d), (g2w, g2_d), (b2w, b2_d),
                           (pwt, pwt_d), (pwg, pwg_d)):
                if t_ is onesrow:
                    nc.gpsimd.dma_start(out=f32r(onesrow[:]), in_=d_[:])
                elif t_ in (xt, w1t):
                    nc.sync.dma_start(out=t_[:], in_=d_[:])
                else:
                    nc.gpsimd.dma_start(out=t_[:], in_=d_[:])

            xp = [pp.tile([128, H], BF16, tag=f"xp{c}", name=f"xp{c}")
                  for c in range(NC)]
            big = [pp.tile([128, TP], BF16, tag=f"big{j}", name=f"big{j}")
                   for j in range(NH)]
            voff = pp.tile([KK, TP], F32, tag="voff")
            s2t = pp.tile([128, 12], F32, tag="s2t")
            wp12 = pp.tile([128, 12], F32, tag="wp12")
            bpp = pp.tile([128, 4], F32, tag="bpp")
            bo = pp.tile([128, 4], F32, tag="bo")
            gs1 = pp.tile([128, 4], F32, tag="gs1")
            bs1 = pp.tile([128, 4], F32, tag="bs1")
            ngs1 = pp.tile([128, 4], F32, tag="ngs1")
            gs2 = pp.tile([128, 4], F32, tag="gs2")
            bs2 = pp.tile([128, 4], F32, tag="bs2")
            diag = [pp.tile([128, 128], BF16, tag=f"diag{j}", name=f"diag{j}")
                    for j in range(12)]
            sca = pp.tile([1, 16], F32, tag="sca")
            invb1 = pp.tile([128, 1], F32, tag="invb1")
            nmb1 = pp.tile([128, 1], F32, tag="nmb1")
            invb2 = pp.tile([128, 1], F32, tag="invb2")
            nmb2 = pp.tile([128, 1], F32, tag="nmb2")
            ccsb = pp.tile([128, 1], F32, tag="ccsb")
            epst = pp.tile([1, 1], F32, tag="epst")

            vtmpP = [pp.tile([1, GW], F32, tag=f"vtmpP{i}", name=f"vtmpP{i}")
                     for i in range(2)]
            for i in range(2):
                nc.gpsimd.dma_start(out=f32r(vtmpP[i][:]), in_=sent_d[:])

            # pinned PSUM accumulator bank: row 0 = S1/S3, row 32 = S2/S4
            sacc1 = pspin.tile([128, 512], F32, tag="sacc", name="sacc1")
            s1p = sacc1[0:1, :]
            s2acc = sacc1[32:33, :]

            # ---------- stage 1b: GEMM1 (HT layout) + S1/S2 stats ----------
            first_s1 = [True]
            for ht in range(NH):
                for n in range(8):
                    s = 512 * n
                    nn = min(512, T - s)
                    g1 = ps.tile([128, 512], F32, tag="ps")
                    nc.tensor.matmul(g1[:, :nn],
                                     lhsT=w1t[:, 128 * ht:128 * ht + 128],
                                     rhs=xt[:, s:s + nn],
                                     start=True, stop=True)
                    nc.scalar.activation(big[ht][:, s:s + nn], g1[:, :nn],
                                         AF.Prelu, bias=0.0, scale=1.0,
                                         alpha=a1)
                    # S1 += sum_h p[h, t] (PE ones-column reduction)
                    nc.tensor.matmul(s1p[:, :nn], lhsT=onescol[:],
                                     rhs=big[ht][:, s:s + nn],
                                     start=first_s1[0],
                                     stop=(ht == NH - 1 and n == 7))
                    first_s1[0] = False
            first_s2 = [True]
            for ht in range(NH):
                for n in range(8):
                    s = 512 * n
                    nn = min(512, T - s)
                    sq = wk.tile([128, 512], BF16, tag="sq", bufs=2)
                    nc.vector.scalar_tensor_tensor(
                        sq[:, :nn], big[ht][:, s:s + nn], 1.0,
                        big[ht][:, s:s + nn], ALU.mult, ALU.mult)
                    nc.tensor.matmul(s2acc[:, :nn], lhsT=onescol[:],
                                     rhs=sq[:, :nn],
                                     start=first_s2[0],
                                     stop=(ht == NH - 1 and n == 7))
                    first_s2[0] = False

            # ---------- stage 1a: GEMM1 (TH layout -> xp windows) ----------
            for c in range(NC):
                g1 = ps.tile([128, 512], F32, tag="ps")
                if c == 0:
                    nc.tensor.matmul(g1[0:126, :], lhsT=xt[:, 0:126],
                                     rhs=w1t[:], start=True, stop=True)
                    t0 = wk.tile([128, H], BF16, tag="t0", bufs=2)
                    nc.scalar.activation(t0[0:126, :], g1[0:126, :], AF.Prelu,
                                         bias=0.0, scale=1.0, alpha=a1)
                    nc.gpsimd.dma_start(out=xp[0][2:128, :], in_=t0[0:126, :])
                    nc.gpsimd.dma_start(out=xp[0][0:1, :], in_=t0[2:3, :])
                    nc.gpsimd.dma_start(out=xp[0][1:2, :], in_=t0[1:2, :])
                elif c < NC - 1:
                    nc.tensor.matmul(g1[:], lhsT=xt[:, 123 * c - 2:123 * c + 126],
                                     rhs=w1t[:], start=True, stop=True)
                    nc.scalar.activation(xp[c][:], g1[:], AF.Prelu,
                                         bias=0.0, scale=1.0, alpha=a1)
                else:
                    nc.vector.memset(xp[c][64:128, :], 0.0)
                    nc.tensor.matmul(g1[0:66, :], lhsT=xt[:, 3934:4000],
                                     rhs=w1t[:], start=True, stop=True)
                    nc.scalar.activation(xp[c][0:66, :], g1[0:66, :], AF.Prelu,
                                         bias=0.0, scale=1.0, alpha=a1)
                    nc.gpsimd.dma_start(out=xp[c][66:67, :], in_=xp[c][64:65, :])
                    nc.gpsimd.dma_start(out=xp[c][67:68, :], in_=xp[c][63:64, :])

            # ---------- stage 2: gLN1 stats + weight folds ----------
            nc.vector.memset(epst[:], EPS)
            nc.vector.tensor_reduce(sca[:, 0:1], s1p[:], AX.X, ALU.add)
            nc.vector.tensor_reduce(sca[:, 1:2], s2acc[:], AX.X, ALU.add)

            def stats_fold(s1c, s2c, mcol, invcol, nmcol, invb, nmb,
                           nt2=NT):
                nc.vector.tensor_scalar(sca[:, mcol:mcol + 1],
                                        sca[:, s1c:s1c + 1], 1.0 / NT, None,
                                        ALU.mult)
                m2c = mcol + 1
                nc.vector.scalar_tensor_tensor(
                    sca[:, m2c:m2c + 1], sca[:, mcol:mcol + 1], 1.0,
                    sca[:, mcol:mcol + 1], ALU.mult, ALU.mult)
                vc = mcol + 2
                nc.vector.scalar_tensor_tensor(
                    sca[:, vc:vc + 1], sca[:, s2c:s2c + 1], 1.0 / nt2,
                    sca[:, m2c:m2c + 1], ALU.mult, ALU.subtract)
                sdc = mcol + 3
                nc.scalar.activation(sca[:, sdc:sdc + 1], sca[:, vc:vc + 1],
                                     AF.Sqrt, bias=epst[:], scale=1.0)
                nc.vector.reciprocal(sca[:, invcol:invcol + 1],
                                     sca[:, sdc:sdc + 1])
                nc.vector.tensor_scalar(sca[:, nmcol:nmcol + 1],
                                        sca[:, mcol:mcol + 1], -1.0, None,
                                        ALU.mult)
                nc.gpsimd.partition_broadcast(invb[:], sca[:, invcol:invcol + 1])
                nc.gpsimd.partition_broadcast(nmb[:], sca[:, nmcol:nmcol + 1])

            stats_fold(0, 1, 2, 6, 7, invb1, nmb1)
            nc.vector.tensor_scalar(gs1[:], g1w[:], invb1[:], None, ALU.mult)
            nc.vector.scalar_tensor_tensor(bs1[:], gs1[:], nmb1[:], b1w[:],
                                           ALU.mult, ALU.add)
            nc.vector.tensor_scalar(ngs1[:], gs1[:], -1.0, None, ALU.mult)
            for ht in range(NH):
                for k in range(KK):
                    col = 3 * ht + k
                    nc.vector.tensor_scalar(s2t[:, col:col + 1],
                                            dww[:, col:col + 1],
                                            gs1[:, ht:ht + 1], None, ALU.mult)
                    nc.vector.tensor_scalar(wp12[:, col:col + 1],
                                            offdw[:, col:col + 1],
                                            gs1[:, ht:ht + 1], None, ALU.mult)
            nc.vector.scalar_tensor_tensor(bpp[:], swdw[:], 1.0, bs1[:],
                                           ALU.mult, ALU.mult)
            nc.vector.scalar_tensor_tensor(bpp[:], bpp[:], 1.0, dwb[:],
                                           ALU.mult, ALU.add)
            nc.vector.scalar_tensor_tensor(bo[:], swo[:], 1.0, bs1[:],
                                           ALU.mult, ALU.mult)
            for ht in range(NH):
                for j in range(KK):
                    col = 3 * ht + j
                    nc.vector.tensor_scalar(diag[col][:], id128[:],
                                            wp12[:, col:col + 1], None,
                                            ALU.mult)

            # ---------- stage 3: offset branch ----------
            for n in range(8):
                s = 512 * n
                nn = min(512, T - s)
                opw = pspin.tile([KK, 512], F32, tag="pss", name=f"opw{n}")
                podcs = []
                for ht in range(NH):
                    d0, d1, d2 = diag[3 * ht], diag[3 * ht + 1], diag[3 * ht + 2]
                    oc = ps.tile([128, 512], F32, tag="ps")
                    nc.tensor.matmul(oc[:, :nn], lhsT=d1[:],
                                     rhs=big[ht][:, s:s + nn],
                                     start=True, stop=False)
                    if n == 0:
                        nc.tensor.matmul(oc[:, 2:nn], lhsT=d0[:],
                                         rhs=big[ht][:, 1:nn - 1],
                                         start=False, stop=False)
                        nc.tensor.matmul(oc[:, 0:1], lhsT=d0[:],
                                         rhs=big[ht][:, 1:2],
                                         start=False, stop=False)
                        nc.tensor.matmul(oc[:, 1:2], lhsT=d0[:],
                                         rhs=big[ht][:, 0:1],
                                         start=False, stop=False)
                    else:
                        nc.tensor.matmul(oc[:, :nn], lhsT=d0[:],
                                         rhs=big[ht][:, s - 1:s - 1 + nn],
                                         start=False, stop=False)
                    if n == 7:
                        nc.tensor.matmul(oc[:, 0:nn - 2], lhsT=d2[:],
                                         rhs=big[ht][:, s + 1:s + nn - 1],
                                         start=False, stop=False)
                        nc.tensor.matmul(oc[:, nn - 2:nn - 1], lhsT=d2[:],
                                         rhs=big[ht][:, T - 1:T],
                                         start=False, stop=False)
                        nc.tensor.matmul(oc[:, nn - 1:nn], lhsT=d2[:],
                                         rhs=big[ht][:, T - 2:T - 1],
                                         start=False, stop=True)
                    else:
                        nc.tensor.matmul(oc[:, :nn], lhsT=d2[:],
                                         rhs=big[ht][:, s + 1:s + 1 + nn],
                                         start=False, stop=True)
                    podc = wk.tile([128, 512], BF16, tag="podc", bufs=4,
                                   name=f"podc{n}_{ht}")
                    nc.scalar.activation(podc[:, :nn], oc[:, :nn], AF.Prelu,
                                         bias=bo[:, ht:ht + 1], scale=1.0,
                                         alpha=aodc)
                    podcs.append(podc)
                for ht in range(NH):
                    nc.tensor.matmul(opw[:, :nn],
                                     lhsT=offpw[:, 3 * ht:3 * ht + 3],
                                     rhs=podcs[ht][:, :nn],
                                     start=(ht == 0), stop=(ht == NH - 1))
                nc.scalar.activation(voff[:, s:s + nn], opw[:KK, :nn],
                                     AF.Prelu, bias=0.0, scale=1.0, alpha=aopc)
                # V = clip(off + dil, 0, 4) + (t mod 123), per block
                nc.vector.tensor_scalar(voff[:, s:s + nn], voff[:, s:s + nn],
                                        dil[:], 0.0, ALU.add, ALU.max)
                nc.vector.scalar_tensor_tensor(
                    voff[:, s:s + nn], voff[:, s:s + nn], 4.0,
                    tmod[:, s:s + nn], ALU.min, ALU.add)
            nc.vector.memset(voff[:, T:TP], 0.0)
            nc.vector.scalar_tensor_tensor(voff[:, T:TP], voff[:, T:TP], 4.0,
                                           tmod[:, T:TP], ALU.min, ALU.add)

            # ---------- stage 4: sampling + combine + prelu2 ----------
            sacc2 = pspin.tile([128, 512], F32, tag="sacc", name="sacc2")
            s3acc = sacc2[0:1, :]
            s4acc = sacc2[32:33, :]
            for g in range(NG):
                cs = [c for c in range(GC * g, GC * (g + 1)) if c < NC]
                ng = len(cs)
                vtmp = vtmpP[g % 2]
                for j, c in enumerate(cs):
                    nc.gpsimd.dma_start(
                        out=f32r(vtmp[0:1, SW * j:SW * j + 3 * TC]),
                        in_=voff[:, TC * c:TC * (c + 1)])
                gp = wk.tile([128, GW], BF16, tag="gp", bufs=2, name=f"gp{g}")
                for j in range(GC):
                    bvp = ps.tile([128, 512], F32, tag="bv", bufs=1, name=f"bv{g}_{j}")
                    nc.tensor.matmul(bvp[:, 0:SW], lhsT=f32r(onesrow[:]),
                                     rhs=f32r(vtmp[0:1, SW * j:SW * (j + 1)]),
                                     start=True, stop=True)
                    zz = wk.tile([128, SW], F32, tag="zz", bufs=3,
                                 name=f"zz{g}_{j}")
                    nc.scalar.activation(zz[:], bvp[:, 0:SW], AF.Abs,
                                         bias=niota[:], scale=1.0)
                    nc.scalar.activation(gp[:, SW * j:SW * (j + 1)],
                                         zz[:], AF.Relu, bias=1.0,
                                         scale=-1.0)
                smt = psm.tile([128, GC * 512], F32, tag="sm", name=f"sm{g}")
                for ht in range(NH):
                    for j, c in enumerate(cs):
                        nc.tensor.matmul(
                            smt[:, 512 * j:512 * j + SW],
                            lhsT=xp[c][:, 128 * ht:128 * ht + 128],
                            rhs=gp[:, SW * j:SW * (j + 1)],
                            start=True, stop=True)
                    c0 = cs[0]
                    qv = big[ht][:, TC * c0:TC * (c0 + ng)]
                    qv3 = qv.rearrange("p (g w) -> p g w", g=ng)
                    smv = smt[:].rearrange("p (g w) -> p g w", g=GC)[:, 0:ng]
                    nc.scalar.activation(qv3, smv[:, :, 0:TC], AF.Identity,
                                         bias=bpp[:, ht:ht + 1],
                                         scale=s2t[:, 3 * ht:3 * ht + 1])
                    nc.vector.scalar_tensor_tensor(
                        qv3, smv[:, :, TC:2 * TC],
                        s2t[:, 3 * ht + 1:3 * ht + 2], qv3,
                        ALU.mult, ALU.add)
                    nc.vector.scalar_tensor_tensor(
                        qv3, smv[:, :, 2 * TC:3 * TC],
                        s2t[:, 3 * ht + 2:3 * ht + 3], qv3,
                        ALU.mult, ALU.add)
                    if cs[-1] == NC - 1:
                        nc.vector.memset(big[ht][:, T:TP], 0.0)
                    # prelu2 on DVE: r = (1-a2)*relu(q); q = a2*q + r
                    rr = wk.tile([128, GC * TC], BF16, tag="rr", bufs=2,
                                 name=f"rr{g}_{ht}")
                    nc.vector.tensor_scalar(rr[:, :TC * ng], qv, 0.0,
                                            (1.0 - a2), ALU.max, ALU.mult)
                    nc.vector.scalar_tensor_tensor(
                        qv, qv, a2, rr[:, :TC * ng], ALU.mult, ALU.add)
                    sq = wk.tile([128, GC * TC], BF16, tag="sq2", bufs=2,
                                 name=f"sqq{g}_{ht}")
                    nc.vector.scalar_tensor_tensor(
                        sq[:, :TC * ng], qv, 1.0, qv, ALU.mult, ALU.mult)
                    nc.tensor.matmul(s4acc[:, 0:TC * ng], lhsT=onescol[:],
                                     rhs=sq[:, :TC * ng],
                                     start=(g == 0 and ht == 0),
                                     stop=(g == NG - 1 and ht == NH - 1))

            # ---------- stage 6: gLN2 folds ----------
            for ht in range(NH):
                for n in range(8):
                    s = 512 * n
                    nn = min(512, T - s)
                    nc.tensor.matmul(s3acc[:, :nn], lhsT=onescol[:],
                                     rhs=big[ht][:, s:s + nn],
                                     start=(ht == 0 and n == 0),
                                     stop=(ht == NH - 1 and n == 7))
            nc.vector.tensor_reduce(sca[:, 8:9], s3acc[:], AX.X, ALU.add)
            nc.vector.tensor_reduce(sca[:, 9:10], s4acc[:, 0:3 * TC], AX.X, ALU.add)
            stats_fold(8, 9, 10, 14, 15, invb2, nmb2)
            nc.vector.tensor_scalar(gs2[:], g2w[:], invb2[:], None, ALU.mult)
            nc.vector.scalar_tensor_tensor(bs2[:], gs2[:], nmb2[:], b2w[:],
                                           ALU.mult, ALU.add)
            ccp = pspin.tile([128, 1], F32, tag="pss", name="ccp")
            for ht in range(NH):
                nc.tensor.matmul(ccp[:], lhsT=pwt[:, 128 * ht:128 * ht + 128],
                                 rhs=bs2[:, ht:ht + 1], start=(ht == 0),
                                 stop=(ht == NH - 1))
            nc.scalar.copy(ccsb[:], ccp[:])

            # ---------- stage 7: GEMM2 in [B, T] + residual ----------
            for n in range(8):
                s = 512 * n
                nn = min(512, T - s)
                g2 = ps.tile([128, 512], F32, tag="ps", name=f"g2_{n}")
                for ht in range(NH):
                    nc.tensor.matmul(g2[:, :nn],
                                     lhsT=pwg[:, 128 * ht:128 * ht + 128],
                                     rhs=big[ht][:, s:s + nn],
                                     start=(ht == 0), stop=(ht == NH - 1))
                yb = wk.tile([128, 512], BF16, tag="yb", bufs=2,
                             name=f"yb{n}")
                nc.scalar.activation(yb[:, :nn], g2[:, :nn], AF.Identity,
                                     bias=ccsb[:], scale=invb2[:])
                osb = wk.tile([128, 512], BF16, tag="osb", bufs=3,
                              name=f"osb{n}")
                nc.vector.scalar_tensor_tensor(
                    osb[:, :nn], yb[:, :nn], 1.0, xt[:, s:s + nn],
                    ALU.mult, ALU.add)
                nc.sync.dma_start(out=out_d[:, s:s + nn], in_=osb[:, :nn])
    return nc


@functools.lru_cache(maxsize=4)
def _prog(alphas):
    nc = _build(*alphas)
    nc.finalize()
    return nc


def _host_consts(conv1_w, off_dw_w, off_pw_w, dw_w, dw_b,
                 norm1_g, norm1_b, norm2_g, norm2_b, pw_w):
    f = np.float32
    c = {}
    c["w1t"] = np.ascontiguousarray(conv1_w.T.astype(f)).astype(BF)
    c["id128"] = np.eye(128, dtype=f).astype(BF)
    c["niota"] = (-np.arange(128, dtype=f)).reshape(128, 1)
    c["ones128"] = np.ones((128, 1), f)
    c["onesrow"] = np.ones((1, 128), f)
    c["onescol"] = np.ones((128, 1), f).astype(BF)
    tm = np.tile(np.arange(TC, dtype=f), NC)[:TP]
    c["tmodTP"] = np.broadcast_to(tm, (KK, TP)).copy()
    c["dil3"] = np.array([[0.0], [2.0], [4.0]], f)

    def blk12(w):
        o = np.zeros((128, 12), f)
        for ht in range(NH):
            for j in range(KK):
                o[:, 3 * ht + j] = w[128 * ht:128 * (ht + 1), j]
        return o

    def blk4(v):
        return np.ascontiguousarray(v.reshape(NH, 128).T.astype(f))

    c["offdw12"] = blk12(off_dw_w.astype(f))
    c["swo4"] = blk4(off_dw_w.sum(1).astype(f))
    opw = np.zeros((128, 12), f)
    for ht in range(NH):
        for k in range(KK):
            opw[:, 3 * ht + k] = off_pw_w[k, 128 * ht:128 * (ht + 1)]
    c["offpwT12"] = opw.astype(BF)
    c["dww12"] = blk12(dw_w.astype(f))
    c["swdw4"] = blk4(dw_w.sum(1).astype(f))
    c["dwb4"] = blk4(dw_b.astype(f))
    c["g1t4"] = blk4(norm1_g.astype(f))
    c["b1t4"] = blk4(norm1_b.astype(f))
    c["g2t4"] = blk4(norm2_g.astype(f))
    c["b2t4"] = blk4(norm2_b.astype(f))
    pwt = np.zeros((128, H), f)
    pwg = np.zeros((128, H), f)
    for ht in range(NH):
        blk = pw_w[:, 128 * ht:128 * (ht + 1)].T
        pwt[:, 128 * ht:128 * (ht + 1)] = blk
        pwg[:, 128 * ht:128 * (ht + 1)] = (
            blk * norm2_g[128 * ht:128 * (ht + 1), None])
    c["pwT4"] = pwt
    c["pwG4"] = pwg.astype(BF)
    c["sentGW"] = np.full((1, GW), 1e6, f)
    return c


def _in_maps(inp):
    x = np.asarray(inp["x"], np.float32)
    consts = _host_consts(np.asarray(inp["conv1_w"]), np.asarray(inp["off_dw_w"]),
                          np.asarray(inp["off_pw_w"]), np.asarray(inp["dw_w"]),
                          np.asarray(inp["dw_b"]), np.asarray(inp["norm1_g"]),
                          np.asarray(inp["norm1_b"]), np.asarray(inp["norm2_g"]),
                          np.asarray(inp["norm2_b"]), np.asarray(inp["pw_w"]))
    return [dict(consts, xt=np.ascontiguousarray(x[m].T).astype(BF))
            for m in range(NCORES)]


def _postprocess_core(out_bt):
    return np.asarray(out_bt).astype(np.float32).T


def kernel(x, conv1_w, prelu1_a, norm1_g, norm1_b,
           off_dw_w, odc_prelu_a, off_pw_w, opc_prelu_a,
           dw_w, dw_b, prelu2_a, norm2_g, norm2_b, pw_w):
    alphas = (float(prelu1_a), float(odc_prelu_a), float(opc_prelu_a),
              float(prelu2_a))
    nc = _prog(alphas)
    in_maps = _in_maps(dict(x=x, conv1_w=conv1_w, norm1_g=norm1_g,
                            norm1_b=norm1_b, off_dw_w=off_dw_w,
                            off_pw_w=off_pw_w, dw_w=dw_w, dw_b=dw_b,
                            norm2_g=norm2_g, norm2_b=norm2_b, pw_w=pw_w))
    res = run_bass_kernel_spmd(nc, in_maps, list(range(NCORES)))
    out = np.stack([_postprocess_core(res.results[m]["out"])
                    for m in range(NCORES)], axis=0)
    return out.astype(np.float32)



# revision 31
# speedup vs baseline: 1.2215x; 1.2215x over previous
"""Trainium2 Bass kernel for nn_DeformableTemporalBlock — v3.

Data-parallel over batch M=8 (one sample per core). Changes vs v2:
  - fp16 datapath everywhere 2-byte (better precision than bf16, same speed).
  - sampling: depthwise tap weights folded into per-tap scaled copies of xp
    (xpk = xp * bw_k, DVE 4x); PE accumulates the 3 taps directly in PSUM;
    single ACT Prelu evacuation fuses bias + prelu2 (scale=-1 absorbs the
    negated G trick). Kills the 2 PSUM-read combine ops + 2-op prelu2 + the
    separate G-broadcast flatten of v2.
  - G weights computed negated in ONE DVE op: gpneg = min(|V-p| - 1, 0).
  - offsets live in [24, 512] layout (8 n-chunks x 3 taps on partitions) so
    prelu/clip ops stop paying 512-wide free-dim cost on 3 partitions.
  - V flatten DMAs dispatched from the idle SP queue (HWDGE) instead of
    gpsimd SWDGE (994ns overhead each), 124-strided tap slots.
  - S4 stats via free accum_out on the DVE square op instead of PE matmuls.
"""

import os
import sys
import functools
import numpy as np

for _p in ("/opt/trn_rl_repo", "/root/.axon_site/_ro/trn_rl_repo"):
    if _p not in sys.path and os.path.isdir(_p):
        sys.path.append(_p)

import concourse.bass as bass
import concourse.mybir as mybir
from concourse import bacc, tile, library_config
from concourse.bass_utils import run_bass_kernel_spmd

F32 = mybir.dt.float32
F16 = mybir.dt.float16
AF = mybir.ActivationFunctionType
ALU = mybir.AluOpType
AX = mybir.AxisListType
NP16 = np.float16

T, B, H, KK = 4000, 128, 512, 3
NH = H // 128          # 4 h-tiles
TC = 123               # t-chunk size (window = TC+5 = 128 = one K tile)
NC = 33                # ceil(T/TC); NC*TC = 4059
TP = NC * TC           # padded t extent 4059
GC = 3                 # chunks per sampling group
NG = (NC + GC - 1) // GC   # 11 groups
TS = 124               # padded per-tap slot width in vtmp/bvp/gpneg
SW = KK * TS           # 372
NT = float(T * H)      # gLN element count
EPS = 1e-8
NCORES = 8
NNC = 8                # 512-wide n-chunks


def _build(a1, aodc, aopc, a2):
    nc = bacc.Bacc("TRN2", target_bir_lowering=False, debug=False)

    def din(name, shape, dt=F32):
        return nc.dram_tensor(name, list(shape), dt, kind="ExternalInput")

    xt_d = din("xt", (B, T), F16)
    w1t_d = din("w1t", (B, H), F16)
    id_d = din("id128", (128, 128), F16)
    idf_d = din("idf32", (128, 128))
    niota_d = din("niota", (128, 1))
    ones128_d = din("ones128", (128, 1))
    onesrow_d = din("onesrow", (1, 128))
    onescol_d = din("onescol", (128, 1), F16)
    tmodA_d = din("tmodA", (128, 512))
    tmodB_d = din("tmodB", (128, 512))
    tmodC_d = din("tmodC", (128, 512))
    dil_d = din("dil128", (128, 1))
    offdw_d = din("offdw12", (128, 12))
    swo_d = din("swo4", (128, 4))
    offpw_d = din("offpwT12", (128, 12), F16)
    dww_d = din("dww12", (128, 12))
    swdw_d = din("swdw4", (128, 4))
    dwb_d = din("dwb4", (128, 4))
    g1_d = din("g1t4", (128, 4))
    b1_d = din("b1t4", (128, 4))
    g2_d = din("g2t4", (128, 4))
    b2_d = din("b2t4", (128, 4))
    pwt_d = din("pwT4", (128, H))
    pwg_d = din("pwG4", (128, H), F16)
    out_d = nc.dram_tensor("out", [B, T], F16, kind="ExternalOutput")

    with tile.TileContext(nc) as tc:
        with tc.tile_pool(name="persist", bufs=1) as pp, \
             tc.tile_pool(name="work", bufs=3) as wk, \
             tc.tile_pool(name="ps", bufs=2, space="PSUM") as ps, \
             tc.tile_pool(name="pspin", bufs=1, space="PSUM") as pspin, \
             tc.tile_pool(name="bv", bufs=2, space="PSUM") as pbv, \
             tc.tile_pool(name="sm", bufs=2, space="PSUM") as psm:
            nc.gpsimd.load_library(library_config.mlp)

            def f32r(ap):
                return ap.bitcast(mybir.dt.float32r)

            # ---- persistent SBUF ----
            xt = pp.tile([B, T], F16, tag="xt")
            w1t = pp.tile([B, H], F16, tag="w1t")
            id128 = pp.tile([128, 128], F16, tag="id128")
            idf32 = pp.tile([128, 128], F32, tag="idf32")
            niota = pp.tile([128, 1], F32, tag="niota")
            ones128 = pp.tile([128, 1], F32, tag="ones128")
            onesrow = pp.tile([1, 128], F32, tag="onesrow")
            onescol = pp.tile([128, 1], F16, tag="onescol")
            tmodA = pp.tile([128, 512], F32, tag="tmodA")
            tmodB = pp.tile([128, 512], F32, tag="tmodB")
            tmodC = pp.tile([128, 512], F32, tag="tmodC")
            dil128 = pp.tile([128, 1], F32, tag="dil128")
            offdw = pp.tile([128, 12], F32, tag="offdw")
            swo = pp.tile([128, 4], F32, tag="swo")
            offpw = pp.tile([128, 12], F16, tag="offpw")
            dww = pp.tile([128, 12], F32, tag="dww")
            swdw = pp.tile([128, 4], F32, tag="swdw")
            dwb = pp.tile([128, 4], F32, tag="dwb")
            g1w = pp.tile([128, 4], F32, tag="g1w")
            b1w = pp.tile([128, 4], F32, tag="b1w")
            g2w = pp.tile([128, 4], F32, tag="g2w")
            b2w = pp.tile([128, 4], F32, tag="b2w")
            pwt = pp.tile([128, H], F32, tag="pwt")
            pwg = pp.tile([128, H], F16, tag="pwg")

            for t_, d_ in ((xt, xt_d), (w1t, w1t_d), (id128, id_d),
                           (idf32, idf_d), (niota, niota_d),
                           (ones128, ones128_d), (onesrow, onesrow_d),
                           (onescol, onescol_d), (tmodA, tmodA_d),
                           (tmodB, tmodB_d), (tmodC, tmodC_d),
                           (dil128, dil_d), (offdw, offdw_d), (swo, swo_d),
                           (offpw, offpw_d), (dww, dww_d), (swdw, swdw_d),
                           (dwb, dwb_d), (g1w, g1_d), (b1w, b1_d),
                           (g2w, g2_d), (b2w, b2_d), (pwt, pwt_d),
                           (pwg, pwg_d)):
                if t_ is onesrow:
                    nc.gpsimd.dma_start(out=f32r(onesrow[:]), in_=d_[:])
                elif t_ in (xt, w1t):
                    nc.sync.dma_start(out=t_[:], in_=d_[:])
                else:
                    nc.gpsimd.dma_start(out=t_[:], in_=d_[:])

            xp = [pp.tile([128, H], F16, tag=f"xp{c}", name=f"xp{c}")
                  for c in range(NC)]
            big = [pp.tile([128, TP], F16, tag=f"big{j}", name=f"big{j}")
                   for j in range(NH)]
            voffA = pp.tile([128, 512], F32, tag="voffA")
            voffB = pp.tile([128, 512], F32, tag="voffB")
            voffC = pp.tile([128, 512], F32, tag="voffC")
            s2t = pp.tile([128, 12], F32, tag="s2t")
            s2tT = pp.tile([12, 128], F32, tag="s2tT")
            brow = [pp.tile([1, H], F32, tag=f"brow{k}", name=f"brow{k}")
                    for k in range(KK)]
            wp12 = pp.tile([128, 12], F32, tag="wp12")
            bpp = pp.tile([128, 4], F32, tag="bpp")
            bo = pp.tile([128, 4], F32, tag="bo")
            gs1 = pp.tile([128, 4], F32, tag="gs1")
            bs1 = pp.tile([128, 4], F32, tag="bs1")
            gs2 = pp.tile([128, 4], F32, tag="gs2")
            bs2 = pp.tile([128, 4], F32, tag="bs2")
            diag = [pp.tile([128, 128], F16, tag=f"diag{j}", name=f"diag{j}")
                    for j in range(12)]
            bwk = [pp.tile([128, H], F16, tag=f"bwk{k}", name=f"bwk{k}")
                   for k in range(KK)]
            sca = pp.tile([1, 16], F32, tag="sca")
            invb1 = pp.tile([128, 1], F32, tag="invb1")
            nmb1 = pp.tile([128, 1], F32, tag="nmb1")
            invb2 = pp.tile([128, 1], F32, tag="invb2")
            nmb2 = pp.tile([128, 1], F32, tag="nmb2")
            ccsb = pp.tile([128, 1], F32, tag="ccsb")
            epst = pp.tile([1, 1], F32, tag="epst")
            s4cols = pp.tile([128, NG * NH], F32, tag="s4cols")
            s4red = pp.tile([128, 1], F32, tag="s4red")

            NVT = 6
            F32R = mybir.dt.float32r
            vtmp = [pp.tile([1, SW], F32R, tag=f"vt{i}", name=f"vt{i}")
                    for i in range(NVT)]
            for i in range(NVT):
                # sentinel columns 123+124k stay 1e6 forever (never DMA'd)
                nc.vector.memset(vtmp[i][:].bitcast(F32), 1.0e6)

            # pinned PSUM: row 0 = S1/S3, row 32 = S2
            sacc1 = pspin.tile([128, 512], F32, tag="sacc", name="sacc1")
            s1p = sacc1[0:1, :]
            s2acc = sacc1[32:33, :]
            # shared bank for small PSUM results
            pmisc = pspin.tile([128, 512], F32, tag="misc", name="pmisc")

            # ---------- stage 1b: GEMM1 (HT layout) + S1/S2 stats ----------
            first_s1 = [True]
            for ht in range(NH):
                for n in range(NNC):
                    s = 512 * n
                    nn = min(512, T - s)
                    g1 = ps.tile([128, 512], F32, tag="ps")
                    nc.tensor.matmul(g1[:, :nn],
                                     lhsT=w1t[:, 128 * ht:128 * ht + 128],
                                     rhs=xt[:, s:s + nn],
                                     start=True, stop=True)
                    nc.scalar.activation(big[ht][:, s:s + nn], g1[:, :nn],
                                         AF.Prelu, bias=0.0, scale=1.0,
                                         alpha=a1)
                    nc.tensor.matmul(s1p[:, :nn], lhsT=onescol[:],
                                     rhs=big[ht][:, s:s + nn],
                                     start=first_s1[0],
                                     stop=(ht == NH - 1 and n == NNC - 1))
                    first_s1[0] = False
            first_s2 = [True]
            for ht in range(NH):
                for n in range(NNC):
                    s = 512 * n
                    nn = min(512, T - s)
                    sq = wk.tile([128, 512], F16, tag="sq", bufs=2)
                    nc.vector.scalar_tensor_tensor(
                        sq[:, :nn], big[ht][:, s:s + nn], 1.0,
                        big[ht][:, s:s + nn], ALU.mult, ALU.mult)
                    nc.tensor.matmul(s2acc[:, :nn], lhsT=onescol[:],
                                     rhs=sq[:, :nn],
                                     start=first_s2[0],
                                     stop=(ht == NH - 1 and n == NNC - 1))
                    first_s2[0] = False

            # ---------- stage 1a: GEMM1 (TH layout -> xp windows) ----------
            for c in range(NC):
                g1 = ps.tile([128, 512], F32, tag="ps")
                if c == 0:
                    nc.tensor.matmul(g1[0:126, :], lhsT=xt[:, 0:126],
                                     rhs=w1t[:], start=True, stop=True)
                    t0 = wk.tile([128, H], F16, tag="t0", bufs=2)
                    nc.scalar.activation(t0[0:126, :], g1[0:126, :], AF.Prelu,
                                         bias=0.0, scale=1.0, alpha=a1)
                    nc.gpsimd.dma_start(out=xp[0][2:128, :], in_=t0[0:126, :])
                    nc.gpsimd.dma_start(out=xp[0][0:1, :], in_=t0[2:3, :])
                    nc.gpsimd.dma_start(out=xp[0][1:2, :], in_=t0[1:2, :])
                elif c < NC - 1:
                    nc.tensor.matmul(g1[:], lhsT=xt[:, 123 * c - 2:123 * c + 126],
                                     rhs=w1t[:], start=True, stop=True)
                    nc.scalar.activation(xp[c][:], g1[:], AF.Prelu,
                                         bias=0.0, scale=1.0, alpha=a1)
                else:
                    nc.vector.memset(xp[c][64:128, :], 0.0)
                    nc.tensor.matmul(g1[0:66, :], lhsT=xt[:, 3934:4000],
                                     rhs=w1t[:], start=True, stop=True)
                    nc.scalar.activation(xp[c][0:66, :], g1[0:66, :], AF.Prelu,
                                         bias=0.0, scale=1.0, alpha=a1)
                    nc.gpsimd.dma_start(out=xp[c][66:67, :], in_=xp[c][64:65, :])
                    nc.gpsimd.dma_start(out=xp[c][67:68, :], in_=xp[c][63:64, :])

            # ---------- stage 2: gLN1 stats + weight folds ----------
            nc.vector.memset(epst[:], EPS)
            nc.vector.tensor_reduce(sca[:, 0:1], s1p[:], AX.X, ALU.add)
            nc.vector.tensor_reduce(sca[:, 1:2], s2acc[:], AX.X, ALU.add)

            def stats_fold(s1c, s2c, mcol, invcol, nmcol, invb, nmb,
                           nt2=NT):
                nc.vector.tensor_scalar(sca[:, mcol:mcol + 1],
                                        sca[:, s1c:s1c + 1], 1.0 / NT, None,
                                        ALU.mult)
                m2c = mcol + 1
                nc.vector.scalar_tensor_tensor(
                    sca[:, m2c:m2c + 1], sca[:, mcol:mcol + 1], 1.0,
                    sca[:, mcol:mcol + 1], ALU.mult, ALU.mult)
                vc = mcol + 2
                nc.vector.scalar_tensor_tensor(
                    sca[:, vc:vc + 1], sca[:, s2c:s2c + 1], 1.0 / nt2,
                    sca[:, m2c:m2c + 1], ALU.mult, ALU.subtract)
                sdc = mcol + 3
                nc.scalar.activation(sca[:, sdc:sdc + 1], sca[:, vc:vc + 1],
                                     AF.Sqrt, bias=epst[:], scale=1.0)
                nc.vector.reciprocal(sca[:, invcol:invcol + 1],
                                     sca[:, sdc:sdc + 1])
                nc.vector.tensor_scalar(sca[:, nmcol:nmcol + 1],
                                        sca[:, mcol:mcol + 1], -1.0, None,
                                        ALU.mult)
                nc.gpsimd.partition_broadcast(invb[:], sca[:, invcol:invcol + 1])
                nc.gpsimd.partition_broadcast(nmb[:], sca[:, nmcol:nmcol + 1])

            stats_fold(0, 1, 2, 6, 7, invb1, nmb1)
            nc.vector.tensor_scalar(gs1[:], g1w[:], invb1[:], None, ALU.mult)
            nc.vector.scalar_tensor_tensor(bs1[:], gs1[:], nmb1[:], b1w[:],
                                           ALU.mult, ALU.add)
            for ht in range(NH):
                for k in range(KK):
                    col = 3 * ht + k
                    nc.vector.tensor_scalar(s2t[:, col:col + 1],
                                            dww[:, col:col + 1],
                                            gs1[:, ht:ht + 1], None, ALU.mult)
                    nc.vector.tensor_scalar(wp12[:, col:col + 1],
                                            offdw[:, col:col + 1],
                                            gs1[:, ht:ht + 1], None, ALU.mult)
            nc.vector.scalar_tensor_tensor(bpp[:], swdw[:], 1.0, bs1[:],
                                           ALU.mult, ALU.mult)
            nc.vector.scalar_tensor_tensor(bpp[:], bpp[:], 1.0, dwb[:],
                                           ALU.mult, ALU.add)
            nc.vector.scalar_tensor_tensor(bo[:], swo[:], 1.0, bs1[:],
                                           ALU.mult, ALU.mult)
            for ht in range(NH):
                for j in range(KK):
                    col = 3 * ht + j
                    nc.vector.tensor_scalar(diag[col][:], id128[:],
                                            wp12[:, col:col + 1], None,
                                            ALU.mult)

            # bwk[k][p, 128*ht + j] = s2t[j, 3*ht + k]  (tap weights along h)
            s2tp = pmisc[0:12, 0:128]
            nc.tensor.matmul(s2tp, lhsT=s2t[:], rhs=idf32[:],
                             start=True, stop=True)
            nc.vector.tensor_scalar(s2tT[:], s2tp, 1.0, None, ALU.mult)
            for k in range(KK):
                for ht in range(NH):
                    r = 3 * ht + k
                    nc.sync.dma_start(
                        out=brow[k][0:1, 128 * ht:128 * ht + 128],
                        in_=s2tT[r:r + 1, :])
                bwp = ps.tile([128, 512], F32, tag="ps", name=f"bwp{k}")
                nc.tensor.matmul(bwp[:], lhsT=f32r(onesrow[:]),
                                 rhs=f32r(brow[k][0:1, :]),
                                 start=True, stop=True)
                nc.vector.tensor_scalar(bwk[k][:], bwp[:], 1.0, None, ALU.mult)

            # ---------- stage 3: offset branch ----------
            # n-chunk n lands at rows 32*(n%3)+k of opw tile n//3
            opwA = pspin.tile([128, 512], F32, tag="sacc", name="opwA")
            opwB = psm.tile([128, 512], F32, tag="sm", name="opwB")
            opwC = psm.tile([128, 512], F32, tag="sm", name="opwC")
            opwT = (opwA, opwB, opwC)
            for t_ in opwT:
                nc.vector.memset(t_[:], 0.0)
            for n in range(NNC):
                s = 512 * n
                nn = min(512, T - s)
                podcs = []
                for ht in range(NH):
                    d0, d1, d2 = diag[3 * ht], diag[3 * ht + 1], diag[3 * ht + 2]
                    oc = ps.tile([128, 512], F32, tag="ps")
                    nc.tensor.matmul(oc[:, :nn], lhsT=d1[:],
                                     rhs=big[ht][:, s:s + nn],
                                     start=True, stop=False)
                    if n == 0:
                        nc.tensor.matmul(oc[:, 2:nn], lhsT=d0[:],
                                         rhs=big[ht][:, 1:nn - 1],
                                         start=False, stop=False)
                        nc.tensor.matmul(oc[:, 0:1], lhsT=d0[:],
                                         rhs=big[ht][:, 1:2],
                                         start=False, stop=False)
                        nc.tensor.matmul(oc[:, 1:2], lhsT=d0[:],
                                         rhs=big[ht][:, 0:1],
                                         start=False, stop=False)
                    else:
                        nc.tensor.matmul(oc[:, :nn], lhsT=d0[:],
                                         rhs=big[ht][:, s - 1:s - 1 + nn],
                                         start=False, stop=False)
                    if n == NNC - 1:
                        nc.tensor.matmul(oc[:, 0:nn - 2], lhsT=d2[:],
                                         rhs=big[ht][:, s + 1:s + nn - 1],
                                         start=False, stop=False)
                        nc.tensor.matmul(oc[:, nn - 2:nn - 1], lhsT=d2[:],
                                         rhs=big[ht][:, T - 1:T],
                                         start=False, stop=False)
                        nc.tensor.matmul(oc[:, nn - 1:nn], lhsT=d2[:],
                                         rhs=big[ht][:, T - 2:T - 1],
                                         start=False, stop=True)
                    else:
                        nc.tensor.matmul(oc[:, :nn], lhsT=d2[:],
                                         rhs=big[ht][:, s + 1:s + 1 + nn],
                                         start=False, stop=True)
                    podc = wk.tile([128, 512], F16, tag="podc", bufs=4,
                                   name=f"podc{n}_{ht}")
                    nc.scalar.activation(podc[:, :nn], oc[:, :nn], AF.Prelu,
                                         bias=bo[:, ht:ht + 1], scale=1.0,
                                         alpha=aodc)
                    podcs.append(podc)
                opwX = opwT[n // 3]
                r0 = 32 * (n % 3)
                for ht in range(NH):
                    nc.tensor.matmul(opwX[r0:r0 + 3, :nn],
                                     lhsT=offpw[:, 3 * ht:3 * ht + 3],
                                     rhs=podcs[ht][:, :nn],
                                     start=(ht == 0), stop=(ht == NH - 1))

            # V = clip(prelu(off) + dil, 0, 4) + (t mod 123)
            voffT = (voffA, voffB, voffC)
            for opwX, voffX, tmodX in ((opwA, voffA, tmodA),
                                       (opwB, voffB, tmodB),
                                       (opwC, voffC, tmodC)):
                nc.scalar.activation(voffX[:], opwX[:], AF.Prelu,
                                     bias=0.0, scale=1.0, alpha=aopc)
                nc.vector.tensor_scalar(voffX[:], voffX[:], dil128[:], 0.0,
                                        ALU.add, ALU.max)
                nc.vector.scalar_tensor_tensor(voffX[:], voffX[:], 4.0,
                                               tmodX[:], ALU.min, ALU.add)

            # flatten V per chunk into single-row [1, 372] tiles (SP HWDGE);
            # tap k occupies cols [124k, 124k+123), col 124k+123 is sentinel.
            # Issued inside the group loop (1 group prefetch) so program
            # order matches slot rotation.
            vts = [vtmp[c % NVT] for c in range(NC)]

            def issue_vdma(c):
                vt = vts[c]
                t0g = TC * c
                n0 = t0g // 512
                a0 = t0g - 512 * n0
                L1 = min(TC, 512 - a0)
                vX = voffT[n0 // 3]
                r0 = 32 * (n0 % 3)
                for k in range(KK):
                    nc.sync.dma_start(
                        out=vt[0:1, TS * k:TS * k + L1],
                        in_=f32r(vX[r0 + k:r0 + k + 1, a0:a0 + L1]))
                if L1 < TC:
                    n1 = n0 + 1
                    vX1 = voffT[n1 // 3]
                    r1 = 32 * (n1 % 3)
                    for k in range(KK):
                        nc.sync.dma_start(
                            out=vt[0:1, TS * k + L1:TS * k + TC],
                            in_=f32r(vX1[r1 + k:r1 + k + 1, 0:TC - L1]))

            # ---------- stage 4: sampling (tap-accumulated) ----------
            for g in range(NG):
                cs = list(range(GC * g, min(GC * (g + 1), NC)))
                if g == 0:
                    for c in cs:
                        issue_vdma(c)
                for c in range(GC * (g + 1), min(GC * (g + 2), NC)):
                    issue_vdma(c)
                gps = []
                for j, c in enumerate(cs):
                    bvp = pbv.tile([128, SW], F32, tag="bv", name=f"bv{c}")
                    nc.tensor.matmul(bvp[:], lhsT=f32r(onesrow[:]),
                                     rhs=vts[c][0:1, :],
                                     start=True, stop=True)
                    zzt = wk.tile([128, SW], F32, tag="zz", bufs=3,
                                  name=f"zz{c}")
                    nc.scalar.activation(zzt[:], bvp[:], AF.Abs,
                                         bias=niota[:], scale=1.0)
                    gpn = wk.tile([128, SW], F16, tag="gpn", bufs=6,
                                  name=f"gpn{c}")
                    nc.vector.tensor_scalar(gpn[:], zzt[:], 1.0, 0.0,
                                            ALU.subtract, ALU.min)
                    gps.append(gpn)
                xpk = []
                for j, c in enumerate(cs):
                    row = []
                    for k in range(KK):
                        xk = wk.tile([128, H], F16, tag="xpk", bufs=12,
                                     name=f"xpk{c}_{k}")
                        nc.vector.tensor_tensor(xk[:], xp[c][:], bwk[k][:],
                                                ALU.mult)
                        row.append(xk)
                    xpk.append(row)
                c0 = cs[0]
                wv = min(TC * len(cs), T - TC * c0)  # valid width for stats
                for ht in range(NH):
                    smt = psm.tile([128, 512], F32, tag="sm",
                                   name=f"sm{g}_{ht}")
                    for j, c in enumerate(cs):
                        for k in range(KK):
                            nc.tensor.matmul(
                                smt[:, TC * j:TC * (j + 1)],
                                lhsT=xpk[j][k][:, 128 * ht:128 * ht + 128],
                                rhs=gps[j][:, TS * k:TS * k + TC],
                                start=(k == 0), stop=(k == KK - 1))
                    zv = big[ht][:, TC * c0:TC * (c0 + len(cs))]
                    nc.scalar.activation(zv, smt[:, 0:TC * len(cs)],
                                         AF.Prelu, bias=bpp[:, ht:ht + 1],
                                         scale=-1.0, alpha=a2)
                    sqs = wk.tile([128, SW], F16, tag="sq4", bufs=2,
                                  name=f"sq4_{g}_{ht}")
                    nc.vector.scalar_tensor_tensor(
                        sqs[:, :wv], zv[:, :wv], 1.0, zv[:, :wv],
                        ALU.mult, ALU.mult,
                        accum_out=s4cols[:, g * NH + ht:g * NH + ht + 1])

            # ---------- stage 6: gLN2 folds ----------
            sacc2 = pspin.tile([128, 512], F32, tag="sacc", name="sacc2")
            s3acc = sacc2[0:1, :]
            for ht in range(NH):
                for n in range(NNC):
                    s = 512 * n
                    nn = min(512, T - s)
                    nc.tensor.matmul(s3acc[:, :nn], lhsT=onescol[:],
                                     rhs=big[ht][:, s:s + nn],
                                     start=(ht == 0 and n == 0),
                                     stop=(ht == NH - 1 and n == NNC - 1))
            nc.vector.tensor_reduce(sca[:, 8:9], s3acc[:], AX.X, ALU.add)
            nc.vector.tensor_reduce(s4red[:], s4cols[:], AX.X, ALU.add)
            s4p = pmisc[0:1, 200:201]
            nc.tensor.matmul(s4p, lhsT=ones128[:], rhs=s4red[:],
                             start=True, stop=True)
            nc.scalar.copy(sca[:, 9:10], s4p)
            stats_fold(8, 9, 10, 14, 15, invb2, nmb2)
            nc.vector.tensor_scalar(gs2[:], g2w[:], invb2[:], None, ALU.mult)
            nc.vector.scalar_tensor_tensor(bs2[:], gs2[:], nmb2[:], b2w[:],
                                           ALU.mult, ALU.add)
            ccp = pmisc[0:128, 256:257]
            for ht in range(NH):
                nc.tensor.matmul(ccp, lhsT=pwt[:, 128 * ht:128 * ht + 128],
                                 rhs=bs2[:, ht:ht + 1], start=(ht == 0),
                                 stop=(ht == NH - 1))
            nc.scalar.copy(ccsb[:], ccp)

            # ---------- stage 7: GEMM2 in [B, T] + residual ----------
            for n in range(NNC):
                s = 512 * n
                nn = min(512, T - s)
                g2 = ps.tile([128, 512], F32, tag="ps", name=f"g2_{n}")
                for ht in range(NH):
                    nc.tensor.matmul(g2[:, :nn],
                                     lhsT=pwg[:, 128 * ht:128 * ht + 128],
                                     rhs=big[ht][:, s:s + nn],
                                     start=(ht == 0), stop=(ht == NH - 1))
                yb = wk.tile([128, 512], F16, tag="yb", bufs=2,
                             name=f"yb{n}")
                nc.scalar.activation(yb[:, :nn], g2[:, :nn], AF.Identity,
                                     bias=ccsb[:], scale=invb2[:])
                osb = wk.tile([128, 512], F16, tag="osb", bufs=3,
                              name=f"osb{n}")
                nc.vector.scalar_tensor_tensor(
                    osb[:, :nn], yb[:, :nn], 1.0, xt[:, s:s + nn],
                    ALU.mult, ALU.add)
                nc.sync.dma_start(out=out_d[:, s:s + nn], in_=osb[:, :nn])
    return nc


@functools.lru_cache(maxsize=4)
def _prog(alphas):
    nc = _build(*alphas)
    nc.finalize()
    return nc


def _host_consts(conv1_w, off_dw_w, off_pw_w, dw_w, dw_b,
                 norm1_g, norm1_b, norm2_g, norm2_b, pw_w):
    f = np.float32
    c = {}
    c["w1t"] = np.ascontiguousarray(conv1_w.T.astype(f)).astype(NP16)
    c["id128"] = np.eye(128, dtype=f).astype(NP16)
    c["idf32"] = np.eye(128, dtype=f)
    c["niota"] = (-np.arange(128, dtype=f)).reshape(128, 1)
    c["ones128"] = np.ones((128, 1), f)
    c["onesrow"] = np.ones((1, 128), f)
    c["onescol"] = np.ones((128, 1), f).astype(NP16)
    tms = [np.zeros((128, 512), f) for _ in range(3)]
    dil = np.zeros((128, 1), f)
    for n in range(NNC):
        for k in range(KK):
            r = 32 * (n % 3) + k
            tms[n // 3][r, :] = (512 * n + np.arange(512)) % TC
            dil[r, 0] = 2.0 * k
    c["tmodA"], c["tmodB"], c["tmodC"] = tms
    c["dil128"] = dil

    def blk12(w):
        o = np.zeros((128, 12), f)
        for ht in range(NH):
            for j in range(KK):
                o[:, 3 * ht + j] = w[128 * ht:128 * (ht + 1), j]
        return o

    def blk4(v):
        return np.ascontiguousarray(v.reshape(NH, 128).T.astype(f))

    c["offdw12"] = blk12(off_dw_w.astype(f))
    c["swo4"] = blk4(off_dw_w.sum(1).astype(f))
    opw = np.zeros((128, 12), f)
    for ht in range(NH):
        for k in range(KK):
            opw[:, 3 * ht + k] = off_pw_w[k, 128 * ht:128 * (ht + 1)]
    c["offpwT12"] = opw.astype(NP16)
    c["dww12"] = blk12(dw_w.astype(f))
    c["swdw4"] = blk4(dw_w.sum(1).astype(f))
    c["dwb4"] = blk4(dw_b.astype(f))
    c["g1t4"] = blk4(norm1_g.astype(f))
    c["b1t4"] = blk4(norm1_b.astype(f))
    c["g2t4"] = blk4(norm2_g.astype(f))
    c["b2t4"] = blk4(norm2_b.astype(f))
    pwt = np.zeros((128, H), f)
    pwg = np.zeros((128, H), f)
    for ht in range(NH):
        blk = pw_w[:, 128 * ht:128 * (ht + 1)].T
        pwt[:, 128 * ht:128 * (ht + 1)] = blk
        pwg[:, 128 * ht:128 * (ht + 1)] = (
            blk * norm2_g[128 * ht:128 * (ht + 1), None])
    c["pwT4"] = pwt
    c["pwG4"] = pwg.astype(NP16)
    return c


def _in_maps(inp):
    x = np.asarray(inp["x"], np.float32)
    consts = _host_consts(np.asarray(inp["conv1_w"]), np.asarray(inp["off_dw_w"]),
                          np.asarray(inp["off_pw_w"]), np.asarray(inp["dw_w"]),
                          np.asarray(inp["dw_b"]), np.asarray(inp["norm1_g"]),
                          np.asarray(inp["norm1_b"]), np.asarray(inp["norm2_g"]),
                          np.asarray(inp["norm2_b"]), np.asarray(inp["pw_w"]))
    return [dict(consts, xt=np.ascontiguousarray(x[m].T).astype(NP16))
            for m in range(NCORES)]


def _postprocess_core(out_bt):
    return np.asarray(out_bt).astype(np.float32).T


def kernel(x, conv1_w, prelu1_a, norm1_g, norm1_b,
           off_dw_w, odc_prelu_a, off_pw_w, opc_prelu_a,
           dw_w, dw_b, prelu2_a, norm2_g, norm2_b, pw_w):
    alphas = (float(prelu1_a), float(odc_prelu_a), float(opc_prelu_a),
              float(prelu2_a))
    nc = _prog(alphas)
    in_maps = _in_maps(dict(x=x, conv1_w=conv1_w, norm1_g=norm1_g,
                            norm1_b=norm1_b, off_dw_w=off_dw_w,
                            off_pw_w=off_pw_w, dw_w=dw_w, dw_b=dw_b,
                            norm2_g=norm2_g, norm2_b=norm2_b, pw_w=pw_w))
    res = run_bass_kernel_spmd(nc, in_maps, list(range(NCORES)))
    out = np.stack([_postprocess_core(res.results[m]["out"])
                    for m in range(NCORES)], axis=0)
    return out.astype(np.float32)
